# revision 8
# baseline (speedup 1.0000x reference)
"""nn_ContrastiveLoss Trainium2 kernel (8 NeuronCores, data-parallel over batch).

Contract: kernel(embeddings=[64,1024,128] f32, labels=[64,1024] int64) -> f32 scalar.

Sharding: batch dim B=64 split as 8 samples per core. Host-side sharding also
packs each sample's rows by label (positives first, then negatives, each
zero-padded to a 128-row multiple) so the device computes the pos x neg hinge
matrix densely instead of the full 1024x1024 with masks. Each core returns
[2, bpc] per-sample (loss, count) partials; host sums across cores and divides
(the all-reduce + final division of the sharding hint).

Device pipeline per sample:
  - DMA packed rows as [128p, TT t, 128d] f32 (row k = t*128 + p)
  - row norms: ACT square -> DVE reduce -> ACT sqrt -> DVE reciprocal
  - cast rows to bf16 (GpSimd copy); diag(rinv) tiles via GpSimd affine_select
  - PE transposes: E_t^T @ diag(rinv_t) -> normalized transposed chunks
    (zero pad rows have rinv=1/eps but x=0, so their columns stay 0)
  - PSUM->SBUF copies cast to bf16 (ACT for pos block, DVE for neg block)
  - PE sim matmuls bf16: sim = et_pos_chunk^T . et_neg -> PSUM fp32
  - fused hinge+reduce, scale-free since sim is fully normalized:
      ACT row-tiles: relu(sim - 0.15) with accum_out
      DVE row-tiles: max(sim, 0.15) summed, then -PADN*0.15 correction
  - per-sample counts from nsq>0 (pad rows have zero norm)
  - GpSimd cross-partition all-reduce -> [2, bpc] output
"""

import sys

if "/opt/trn_rl_repo" not in sys.path:
    sys.path.insert(0, "/opt/trn_rl_repo")

from contextlib import ExitStack

import numpy as np

import concourse.bass as bass
import concourse.bacc as bacc
import concourse.mybir as mybir
import concourse.tile as tile
from concourse import bass_isa, bass_utils

F32 = mybir.dt.float32
BF16 = mybir.dt.bfloat16
AF = mybir.ActivationFunctionType
ALU = mybir.AluOpType

P = 128      # SBUF partitions
D = 128      # embedding dim
N = 1024     # rows per sample
B = 64       # full batch
NCORES = 8
BPC = B // NCORES
THRESH = 0.5 - 0.35   # margin threshold 0.15
EPS = 1e-6


def _kernel_body(ctx, tc, emb_ap, out_ap, bpc, padp, padn):
    nc = tc.nc
    tp, tn = padp // P, padn // P
    tt = tp + tn

    const_pool = ctx.enter_context(tc.tile_pool(name="const", bufs=1))
    epool = ctx.enter_context(tc.tile_pool(name="epool", bufs=2))
    etpool = ctx.enter_context(tc.tile_pool(name="etpool", bufs=2))
    diagpool = ctx.enter_context(tc.tile_pool(name="diagpool", bufs=2))
    small = ctx.enter_context(tc.tile_pool(name="small", bufs=2))
    acc_pool = ctx.enter_context(tc.tile_pool(name="acc", bufs=1))
    tr_psum = ctx.enter_context(tc.tile_pool(name="trps", bufs=2, space="PSUM"))
    sim_psum = ctx.enter_context(tc.tile_pool(name="simps", bufs=2, space="PSUM"))

    neg_thr = const_pool.tile([P, 1], F32)
    nc.gpsimd.memset(neg_thr[:], -THRESH)
    eps2 = const_pool.tile([P, 1], F32)
    nc.gpsimd.memset(eps2[:], EPS * EPS)

    red_all = acc_pool.tile([P, 3, bpc], F32)   # rows: S, n_pos, n_neg
    nsq_all = acc_pool.tile([P, bpc, tt], F32)
    slot_all = acc_pool.tile([P, bpc, tp], F32)

    for b in range(bpc):
        e_nat = epool.tile([P, tt, D], F32, tag="e_nat")
        nc.sync.dma_start(e_nat[:], emb_ap[b].rearrange("(t p) d -> p t d", p=P))

        esq = epool.tile([P, tt, D], F32, tag="esq")
        nc.scalar.activation(esq[:], e_nat[:], AF.Square)
        nsq = nsq_all[:, b, :]
        nc.vector.tensor_reduce(nsq, esq[:], axis=mybir.AxisListType.X,
                                op=ALU.add)
        # r = sqrt(nsq + eps^2) folds in the max(r, eps) clamp (pad rows)
        r_ = small.tile([P, tt], F32, tag="r_")
        nc.scalar.activation(r_[:], nsq, AF.Sqrt, bias=eps2[:])
        rinv = small.tile([P, tt], F32, tag="rinv")
        nc.vector.reciprocal(rinv[:], r_[:])

        e_bf = epool.tile([P, tt, D], BF16, tag="e_bf")
        nc.gpsimd.tensor_copy(e_bf[:, 0:tp, :], e_nat[:, 0:tp, :])
        nc.vector.tensor_copy(e_bf[:, tp:tt, :], e_nat[:, tp:tt, :])
        diagall = diagpool.tile([P, tt, D], BF16, tag="diag")
        nc.gpsimd.affine_select(
            diagall[:], rinv[:].unsqueeze(2).broadcast_to([P, tt, D]),
            pattern=[[0, tt], [-1, D]], compare_op=ALU.is_equal, fill=0.0,
            base=0, channel_multiplier=1,
        )

        # normalized transposes: chunk^T @ diag(rinv_chunk) -> fp32 PSUM
        ps_p = tr_psum.tile([P, padp], F32, tag="trps")
        ps_n = tr_psum.tile([P, padn], F32, tag="trps")
        for t in range(tp):
            nc.tensor.matmul(ps_p[:, bass.ts(t, P)], lhsT=e_bf[:, t, :],
                             rhs=diagall[:, t, :], start=True, stop=True)
        for t in range(tn):
            nc.tensor.matmul(ps_n[:, bass.ts(t, P)], lhsT=e_bf[:, tp + t, :],
                             rhs=diagall[:, tp + t, :], start=True, stop=True)
        et_p = etpool.tile([P, padp], BF16, tag="et_p")
        nc.scalar.copy(et_p[:], ps_p[:])
        et_n = etpool.tile([P, padn], BF16, tag="et_n")
        nc.vector.tensor_copy(et_n[:], ps_n[:])

        # sim matmuls + fused scale-free hinge reduction
        slot = slot_all[:, b, :]
        for mt in range(tp):
            sim_ps = sim_psum.tile([P, padn], F32, tag="simps")
            for j0 in range(0, padn, 512):
                jw = min(512, padn - j0)
                nc.tensor.matmul(sim_ps[:, j0:j0 + jw],
                                 lhsT=et_p[:, bass.ts(mt, P)],
                                 rhs=et_n[:, j0:j0 + jw],
                                 start=True, stop=True)
            if mt % 2 == 1:
                nc.scalar.activation(sim_ps[:], sim_ps[:], AF.Relu,
                                     bias=neg_thr[:],
                                     accum_out=slot[:, mt:mt + 1])
            else:
                nc.vector.tensor_scalar(sim_ps[:], sim_ps[:], THRESH, None,
                                        ALU.max, ALU.add,
                                        accum_out=slot[:, mt:mt + 1])

    # S per sample: one batched reduce over slots, then remove the DVE
    # max-trick offset (ndve slots counted padn*t each)
    ndve = len(range(0, tp, 2))
    nc.vector.tensor_reduce(red_all[:, 0, :], slot_all[:],
                            axis=mybir.AxisListType.X, op=ALU.add)
    s_fix = acc_pool.tile([P, bpc], F32)
    nc.vector.tensor_scalar_sub(s_fix[:], red_all[:, 0, :],
                                float(ndve) * float(padn) * THRESH)
    nc.vector.tensor_copy(red_all[:, 0, :], s_fix[:])

    # counts: real rows have nsq > 0 (pad rows are all-zero)
    live = acc_pool.tile([P, bpc, tt], F32)
    nc.vector.tensor_scalar(live[:], nsq_all[:], 0.0, None, ALU.is_gt)
    nc.vector.tensor_reduce(red_all[:, 1, :], live[:, :, 0:tp],
                            axis=mybir.AxisListType.X, op=ALU.add)
    nc.vector.tensor_reduce(red_all[:, 2, :], live[:, :, tp:tt],
                            axis=mybir.AxisListType.X, op=ALU.add)

    redr = acc_pool.tile([P, 3, bpc], F32)
    nc.gpsimd.partition_all_reduce(redr[:], red_all[:], channels=P,
                                   reduce_op=bass_isa.ReduceOp.add)
    sr, npr, nnr = redr[:, 0, :], redr[:, 1, :], redr[:, 2, :]

    # S is already 0 for invalid samples (no pos or no neg rows -> zero sim),
    # so loss = S / max(n_neg, 1); count = n_pos * (n_neg > 0)
    nnc = small.tile([P, bpc], F32, tag="nnc")
    nc.vector.tensor_scalar_max(nnc[:], nnr, 1.0)
    nninv = small.tile([P, bpc], F32, tag="nninv")
    nc.vector.reciprocal(nninv[:], nnc[:])
    lossv = small.tile([P, bpc], F32, tag="lossv")
    nc.vector.tensor_mul(lossv[:], sr, nninv[:])
    vn = small.tile([P, bpc], F32, tag="vn")
    nc.vector.tensor_scalar(vn[:], nnr, 0.5, None, ALU.is_gt)
    cntv = small.tile([P, bpc], F32, tag="cntv")
    nc.vector.tensor_mul(cntv[:], npr, vn[:])

    nc.sync.dma_start(out_ap[0:1, :], lossv[0:1, :])
    nc.sync.dma_start(out_ap[1:2, :], cntv[0:1, :])


_NC_CACHE = {}


def _build(padp, padn):
    key = (BPC, NCORES, padp, padn)
    if key in _NC_CACHE:
        return _NC_CACHE[key]
    nc = bacc.Bacc("TRN2", target_bir_lowering=False, debug=False,
                   num_devices=NCORES)
    emb = nc.dram_tensor("emb", [BPC, padp + padn, D], F32,
                         kind="ExternalInput")
    out = nc.dram_tensor("out", [2, BPC], F32, kind="ExternalOutput")
    with tile.TileContext(nc) as tc:
        with ExitStack() as ctx:
            _kernel_body(ctx, tc, emb.ap(), out.ap(), BPC, padp, padn)
    nc.compile()
    _NC_CACHE[key] = nc
    return nc


def _pack(emb, labels):
    """Per-sample label packing: pos rows, zero pad, neg rows, zero pad."""
    npos = (labels == 1).sum(axis=1)
    nneg = (labels == 0).sum(axis=1)
    padp = max(P, int(-(-npos.max() // P)) * P)
    padn = max(P, int(-(-nneg.max() // P)) * P)
    packed = np.zeros((B, padp + padn, D), np.float32)
    for b in range(B):
        pos_idx = np.nonzero(labels[b] == 1)[0]
        neg_idx = np.nonzero(labels[b] == 0)[0]
        packed[b, :len(pos_idx)] = emb[b, pos_idx]
        packed[b, padp:padp + len(neg_idx)] = emb[b, neg_idx]
    return packed, padp, padn


def kernel(embeddings: np.ndarray, labels: np.ndarray,
           _want_results=False, _trace=False) -> np.ndarray:
    emb = np.ascontiguousarray(embeddings, dtype=np.float32)
    lab = np.asarray(labels)
    assert emb.shape == (B, N, D) and lab.shape == (B, N)

    packed, padp, padn = _pack(emb, lab)
    nc = _build(padp, padn)
    in_maps = [{"emb": packed[c * BPC:(c + 1) * BPC]} for c in range(NCORES)]
    res = bass_utils.run_bass_kernel_spmd(nc, in_maps,
                                          core_ids=list(range(NCORES)),
                                          trace=_trace)
    loss_sum = 0.0
    count = 0.0
    for c in range(NCORES):
        o = res.results[c]["out"]
        loss_sum += float(o[0].sum())
        count += float(o[1].sum())
    ans = np.float32(loss_sum) / np.float32(max(count, 1.0))
    if _want_results:
        return np.float32(ans), res
    return np.float32(ans)
